# revision 13
# baseline (speedup 1.0000x reference)
"""Trainium2 Bass kernel for CMPNEncoder functional-group embedding (v4).

out = func_save_init + A @ W,  A[s,:] = sum_a count_s[a] * f_atoms[a,:].

Per core (atoms sharded 8 ways): stream only the *referenced* atom rows
(~80% of the shard) paired with their per-segment count rows, reduce via
one fp8 matmul per 128-row tile into a PSUM [100,133] accumulator, then
apply W on-device.

v4 changes vs v3:
  - table rows shipped as fp8 e3m4 (1 B/elem, measured end-to-end rel err
    ~1.3e-2 < 2e-2 gate) instead of bf16 hi/lo pairs (4x fewer bytes,
    2x fewer matmuls).
  - counts shipped as fp8 e3m4 (exact for counts <= 32; measured max 3).
  - counts+table packed per tile into one DRAM tensor [128, ntiles*233]
    so each chunk is ONE large per-partition-contiguous DMA (>1 MiB =>
    near line rate, ~358 GB/s HBM/NC).
Host sums the per-core [100,300] partials (unshard) and adds
func_save_init.
"""

import sys

sys.path.insert(0, "/opt/trn_rl_repo")

import ml_dtypes
import numpy as np

import concourse.bacc as bacc
import concourse.mybir as mybir
from concourse.bass_utils import run_bass_kernel_spmd
from concourse.tile import TileContext

N_ATOMS = 400_000
FDIM = 133
HID = 300
NSEG = 100
N_CORES = 8
ROWS_PER_CORE = N_ATOMS // N_CORES
TW = 234                                  # padded tile-row slot (fp8 bytes)
TOFF = 100                                # table offset within slot (4B-aligned)
CHUNK = 64                                # tiles per streamed DMA chunk


def _round_up(x, m):
    return (x + m - 1) // m * m


def _chunk_sizes(ntiles, chunk=CHUNK):
    """Full-size chunks first, tapered at the END: the stream is DMA-bound,
    so total time ~= last-chunk-sem + PE work left after it. Small final
    chunks minimize that trailing PE work."""
    taper = [32, 16, 8, 4, 2]
    rem = ntiles - sum(taper)
    if rem <= 0:  # tiny problem fallback
        return [ntiles]
    sizes = [chunk] * (rem // chunk)
    if rem % chunk:
        sizes.append(rem % chunk)
    return sizes + taper


def build_nc(ntiles, fdim=FDIM, hid=HID, nseg=NSEG):
    f32, bf16, fp8 = (mybir.dt.float32, mybir.dt.bfloat16,
                      mybir.dt.float8e3)

    nc = bacc.Bacc("TRN2", target_bir_lowering=False, debug=False)

    comb = nc.declare_dram_parameter("comb", [128, ntiles * TW], fp8,
                                     isOutput=False)
    wmat = nc.declare_dram_parameter("wmat", [fdim, hid], bf16,
                                     isOutput=False)
    ident_d = nc.declare_dram_parameter("ident", [nseg, nseg], bf16,
                                        isOutput=False)
    out_d = nc.declare_dram_parameter("out", [nseg // 2, 2 * hid], bf16,
                                      isOutput=True)

    sizes = _chunk_sizes(ntiles)

    with TileContext(nc) as tc:
        with (
            tc.tile_pool(name="const", bufs=1) as cpool,
            tc.tile_pool(name="stream", bufs=6) as spool,
            tc.tile_pool(name="psA", bufs=1, space="PSUM") as psA,
            tc.tile_pool(name="psT", bufs=1, space="PSUM") as psT,
            tc.tile_pool(name="sb2", bufs=1) as sb2,
        ):
            a_ps = psA.tile([nseg, fdim], f32, tag="A")

            # Issue ALL stream DMAs up front (the first one is the critical
            # path to the first matmul); consts (only needed at the epilogue)
            # go to the sync queue after the stream is rolling.
            chunks = []
            t0 = 0
            for g in sizes:
                ft = spool.tile([128, CHUNK * TW], fp8, tag="f")
                nc.sync.dma_start(out=ft[:, 0:g * TW],
                                  in_=comb[:, t0 * TW:(t0 + g) * TW])
                chunks.append((ft, g))
                t0 += g
                if len(chunks) == 1:
                    ident_t = cpool.tile([nseg, nseg], bf16, tag="ident")
                    nc.scalar.dma_start(out=ident_t[:, :], in_=ident_d[:, :])
                    wa_t = cpool.tile([128, hid], bf16, tag="wa")
                    nc.scalar.dma_start(out=wa_t[:, :], in_=wmat[0:128, :])
                    wb_t = cpool.tile([fdim - 128, hid], bf16, tag="wb")
                    nc.scalar.dma_start(out=wb_t[:, :], in_=wmat[128:fdim, :])

            tglob = 0
            for ft, g in chunks:
                for j in range(g):
                    nc.tensor.matmul(
                        out=a_ps[:, :],
                        lhsT=ft[:, j * TW:j * TW + nseg],
                        rhs=ft[:, j * TW + TOFF:j * TW + TOFF + fdim],
                        start=(tglob == 0),
                        stop=(tglob == ntiles - 1),
                    )
                    tglob += 1

            # Epilogue: A[100,133] -> A^T via PE transpose -> out = A @ W.
            a_sb = sb2.tile([nseg, fdim], bf16, tag="a_sb")
            nc.vector.tensor_copy(out=a_sb[:, :], in_=a_ps[:, :])
            t1_ps = psT.tile([128, nseg], bf16, tag="t1")
            nc.tensor.transpose(out=t1_ps[:, :], in_=a_sb[:, 0:128],
                                identity=ident_t[:, :])
            at1_sb = sb2.tile([128, nseg], bf16, tag="at1")
            nc.vector.tensor_copy(out=at1_sb[:, :], in_=t1_ps[:, :])
            t2_ps = psT.tile([fdim - 128, nseg], bf16, tag="t2")
            nc.tensor.transpose(out=t2_ps[:, :], in_=a_sb[:, 128:fdim],
                                identity=ident_t[:, :])
            at2_sb = sb2.tile([fdim - 128, nseg], bf16, tag="at2")
            nc.scalar.copy(out=at2_sb[:, :], in_=t2_ps[:, :])

            # out laid out [50, 600]: segments 0-49 in cols 0:300, segments
            # 50-99 in cols 300:600 (half the output-DMA descriptors; host
            # unpacks). The W matmuls write each half directly via lhsT
            # free-dim slices -- no PSUM partition-offset reads.
            half = nseg // 2
            o_ps = psT.tile([half, 2 * hid], f32, tag="o")
            nc.tensor.matmul(out=o_ps[:, 0:hid], lhsT=at1_sb[:, 0:half],
                             rhs=wa_t[:, :], start=True, stop=False)
            nc.tensor.matmul(out=o_ps[:, 0:hid], lhsT=at2_sb[:, 0:half],
                             rhs=wb_t[:, :], start=False, stop=True)
            nc.tensor.matmul(out=o_ps[:, hid:2 * hid],
                             lhsT=at1_sb[:, half:nseg], rhs=wa_t[:, :],
                             start=True, stop=False)
            nc.tensor.matmul(out=o_ps[:, hid:2 * hid],
                             lhsT=at2_sb[:, half:nseg], rhs=wb_t[:, :],
                             start=False, stop=True)
            o_sb = sb2.tile([half, 2 * hid], bf16, tag="o_sb")
            nc.vector.tensor_copy(out=o_sb[:, :], in_=o_ps[:, :])
            nc.sync.dma_start(out=out_d[:, :], in_=o_sb[:, :])

    nc.compile()
    return nc


def prepare_inputs(f_atoms, W, func2atom, mapping,
                   n_cores=N_CORES, rows_tbl=ROWS_PER_CORE, nseg=NSEG):
    fdim = f_atoms.shape[1]
    flat = func2atom.astype(np.int64).ravel()
    seg = np.repeat(mapping.astype(np.int64), func2atom.shape[1])
    valid = flat > 0
    atom = flat[valid] - 1
    seg = seg[valid]
    core = atom // rows_tbl
    local = atom % rows_tbl

    # Per-core count matrices over the core's referenced (compacted) rows.
    percore = []
    for c in range(n_cores):
        m = core == c
        cnt = np.zeros((rows_tbl, nseg), dtype=np.float32)
        np.add.at(cnt, (local[m], seg[m]), 1.0)
        ref = np.flatnonzero(cnt.any(axis=1))
        percore.append((ref, cnt[ref]))

    rows_pad = _round_up(max(len(r) for r, _ in percore), 128)
    ntiles = rows_pad // 128
    ident = np.eye(nseg, dtype=ml_dtypes.bfloat16)
    w_bf = W.astype(ml_dtypes.bfloat16)

    in_maps = []
    for c in range(n_cores):
        ref, cnt = percore[c]
        n = len(ref)
        assert cnt.max() <= 32.0  # fp8 e3m4 is exact for small ints
        rows = f_atoms[c * rows_tbl:(c + 1) * rows_tbl][ref]
        comb = np.zeros((128, ntiles, TW), dtype=ml_dtypes.float8_e3m4)
        tbl = np.zeros((128 * ntiles, fdim), dtype=ml_dtypes.float8_e3m4)
        tbl[:n] = rows.astype(ml_dtypes.float8_e3m4)
        cp = np.zeros((128 * ntiles, nseg), dtype=ml_dtypes.float8_e3m4)
        cp[:n] = cnt.astype(ml_dtypes.float8_e3m4)
        # slot (p, t) holds compacted row p*ntiles + t so each partition's
        # DRAM stream is fully contiguous
        comb[:, :, :nseg] = cp.reshape(128, ntiles, nseg)
        comb[:, :, TOFF:TOFF + fdim] = tbl.reshape(128, ntiles, fdim)
        in_maps.append({
            "comb": comb.reshape(128, ntiles * TW),
            "wmat": w_bf,
            "ident": ident,
        })
    return in_maps, ntiles


_CACHE = {}


def kernel(f_atoms, W, func2atom, mapping, func_save_init, _trace=False):
    in_maps, ntiles = prepare_inputs(f_atoms, W, func2atom, mapping)
    if ntiles not in _CACHE:
        _CACHE[ntiles] = build_nc(ntiles)
    nc = _CACHE[ntiles]
    res = run_bass_kernel_spmd(nc, in_maps, list(range(N_CORES)),
                               trace=_trace)
    packed = sum(r["out"].astype(np.float32) for r in res.results)
    partial = np.concatenate([packed[:, :HID], packed[:, HID:]], axis=0)
    out = func_save_init.astype(np.float32) + partial
    if _trace:
        kernel.last_exec_time_ns = res.exec_time_ns
    return out


# revision 14
# speedup vs baseline: 1.0342x; 1.0342x over previous
"""Trainium2 Bass kernel for CMPNEncoder functional-group embedding (v4).

out = func_save_init + A @ W,  A[s,:] = sum_a count_s[a] * f_atoms[a,:].

Per core (atoms sharded 8 ways): stream only the *referenced* atom rows
(~80% of the shard) paired with their per-segment count rows, reduce via
one fp8 matmul per 128-row tile into a PSUM [100,133] accumulator, then
apply W on-device.

v4 changes vs v3:
  - table rows shipped as fp8 e3m4 (1 B/elem, measured end-to-end rel err
    ~1.3e-2 < 2e-2 gate) instead of bf16 hi/lo pairs (4x fewer bytes,
    2x fewer matmuls).
  - counts shipped as fp8 e3m4 (exact for counts <= 32; measured max 3).
  - counts+table packed per tile into one DRAM tensor [128, ntiles*233]
    so each chunk is ONE large per-partition-contiguous DMA (>1 MiB =>
    near line rate, ~358 GB/s HBM/NC).
Host sums the per-core [100,300] partials (unshard) and adds
func_save_init.
"""

import sys

sys.path.insert(0, "/opt/trn_rl_repo")

import ml_dtypes
import numpy as np

import concourse.bacc as bacc
import concourse.mybir as mybir
from concourse.bass_utils import run_bass_kernel_spmd
from concourse.tile import TileContext

N_ATOMS = 400_000
FDIM = 133
HID = 300
NSEG = 100
N_CORES = 8
ROWS_PER_CORE = N_ATOMS // N_CORES
TW = 236                                  # padded tile-row slot (fp8 bytes)
TOFF = 100                                # table offset within slot (4B-aligned)
CHUNK = 64                                # tiles per streamed DMA chunk


def _round_up(x, m):
    return (x + m - 1) // m * m


def _chunk_sizes(ntiles, chunk=CHUNK):
    """Full-size chunks first, tapered at the END: the stream is DMA-bound,
    so total time ~= last-chunk-sem + PE work left after it. Small final
    chunks minimize that trailing PE work."""
    taper = [32, 16, 8]
    rem = ntiles - sum(taper)
    if rem <= 0:  # tiny problem fallback
        return [ntiles]
    sizes = [chunk] * (rem // chunk)
    if rem % chunk:
        sizes.append(rem % chunk)
    return sizes + taper


def build_nc(ntiles, fdim=FDIM, hid=HID, nseg=NSEG):
    f32, bf16, fp8 = (mybir.dt.float32, mybir.dt.bfloat16,
                      mybir.dt.float8e3)

    nc = bacc.Bacc("TRN2", target_bir_lowering=False, debug=False)

    comb = nc.declare_dram_parameter("comb", [128, ntiles * TW], fp8,
                                     isOutput=False)
    wmat = nc.declare_dram_parameter("wmat", [fdim, hid], bf16,
                                     isOutput=False)
    ident_d = nc.declare_dram_parameter("ident", [nseg, nseg], bf16,
                                        isOutput=False)
    out_d = nc.declare_dram_parameter("out", [nseg, hid], f32, isOutput=True)

    sizes = _chunk_sizes(ntiles)

    with TileContext(nc) as tc:
        with (
            tc.tile_pool(name="const", bufs=1) as cpool,
            tc.tile_pool(name="stream", bufs=5) as spool,
            tc.tile_pool(name="psA", bufs=1, space="PSUM") as psA,
            tc.tile_pool(name="psT", bufs=1, space="PSUM") as psT,
            tc.tile_pool(name="sb2", bufs=1) as sb2,
        ):
            a_ps = psA.tile([nseg, fdim], f32, tag="A")

            # Issue ALL stream DMAs up front (the first one is the critical
            # path to the first matmul); consts (only needed at the epilogue)
            # go to the sync queue after the stream is rolling.
            chunks = []
            t0 = 0
            for g in sizes:
                ft = spool.tile([128, CHUNK * TW], fp8, tag="f")
                nc.sync.dma_start(out=ft[:, 0:g * TW],
                                  in_=comb[:, t0 * TW:(t0 + g) * TW])
                chunks.append((ft, g))
                t0 += g
                if len(chunks) == 1:
                    ident_t = cpool.tile([nseg, nseg], bf16, tag="ident")
                    nc.scalar.dma_start(out=ident_t[:, :], in_=ident_d[:, :])
                    wa_t = cpool.tile([128, hid], bf16, tag="wa")
                    nc.scalar.dma_start(out=wa_t[:, :], in_=wmat[0:128, :])
                    wb_t = cpool.tile([fdim - 128, hid], bf16, tag="wb")
                    nc.scalar.dma_start(out=wb_t[:, :], in_=wmat[128:fdim, :])

            tglob = 0
            for ft, g in chunks:
                for j in range(g):
                    nc.tensor.matmul(
                        out=a_ps[:, :],
                        lhsT=ft[:, j * TW:j * TW + nseg],
                        rhs=ft[:, j * TW + TOFF:j * TW + TOFF + fdim],
                        start=(tglob == 0),
                        stop=(tglob == ntiles - 1),
                    )
                    tglob += 1

            # Epilogue: A[100,133] -> A^T via PE transpose -> out = A @ W.
            a_sb = sb2.tile([nseg, fdim], bf16, tag="a_sb")
            nc.scalar.copy(out=a_sb[:, :], in_=a_ps[:, :])
            t1_ps = psT.tile([128, nseg], bf16, tag="t1")
            nc.tensor.transpose(out=t1_ps[:, :], in_=a_sb[:, 0:128],
                                identity=ident_t[:, :])
            at1_sb = sb2.tile([128, nseg], bf16, tag="at1")
            nc.vector.tensor_copy(out=at1_sb[:, :], in_=t1_ps[:, :])
            t2_ps = psT.tile([fdim - 128, nseg], bf16, tag="t2")
            nc.tensor.transpose(out=t2_ps[:, :], in_=a_sb[:, 128:fdim],
                                identity=ident_t[:, :])
            at2_sb = sb2.tile([fdim - 128, nseg], bf16, tag="at2")
            nc.scalar.copy(out=at2_sb[:, :], in_=t2_ps[:, :])

            o_ps = psT.tile([nseg, hid], f32, tag="o")
            nc.tensor.matmul(out=o_ps[:, :], lhsT=at1_sb[:, :], rhs=wa_t[:, :],
                             start=True, stop=False)
            nc.tensor.matmul(out=o_ps[:, :], lhsT=at2_sb[:, :], rhs=wb_t[:, :],
                             start=False, stop=True)
            o_sb = sb2.tile([nseg, hid], f32, tag="o_sb")
            hh = hid // 2
            nc.vector.tensor_copy(out=o_sb[:, 0:hh], in_=o_ps[:, 0:hh])
            nc.scalar.copy(out=o_sb[:, hh:hid], in_=o_ps[:, hh:hid])
            nc.sync.dma_start(out=out_d[:, :], in_=o_sb[:, :])

    nc.compile()
    return nc


def prepare_inputs(f_atoms, W, func2atom, mapping,
                   n_cores=N_CORES, rows_tbl=ROWS_PER_CORE, nseg=NSEG):
    fdim = f_atoms.shape[1]
    flat = func2atom.astype(np.int64).ravel()
    seg = np.repeat(mapping.astype(np.int64), func2atom.shape[1])
    valid = flat > 0
    atom = flat[valid] - 1
    seg = seg[valid]
    core = atom // rows_tbl
    local = atom % rows_tbl

    # Per-core count matrices over the core's referenced (compacted) rows.
    percore = []
    for c in range(n_cores):
        m = core == c
        cnt = np.zeros((rows_tbl, nseg), dtype=np.float32)
        np.add.at(cnt, (local[m], seg[m]), 1.0)
        ref = np.flatnonzero(cnt.any(axis=1))
        percore.append((ref, cnt[ref]))

    rows_pad = _round_up(max(len(r) for r, _ in percore), 128)
    ntiles = rows_pad // 128
    ident = np.eye(nseg, dtype=ml_dtypes.bfloat16)
    w_bf = W.astype(ml_dtypes.bfloat16)

    in_maps = []
    for c in range(n_cores):
        ref, cnt = percore[c]
        n = len(ref)
        assert cnt.max() <= 32.0  # fp8 e3m4 is exact for small ints
        rows = f_atoms[c * rows_tbl:(c + 1) * rows_tbl][ref]
        comb = np.zeros((128, ntiles, TW), dtype=ml_dtypes.float8_e3m4)
        tbl = np.zeros((128 * ntiles, fdim), dtype=ml_dtypes.float8_e3m4)
        tbl[:n] = rows.astype(ml_dtypes.float8_e3m4)
        cp = np.zeros((128 * ntiles, nseg), dtype=ml_dtypes.float8_e3m4)
        cp[:n] = cnt.astype(ml_dtypes.float8_e3m4)
        # slot (p, t) holds compacted row p*ntiles + t so each partition's
        # DRAM stream is fully contiguous
        comb[:, :, :nseg] = cp.reshape(128, ntiles, nseg)
        comb[:, :, TOFF:TOFF + fdim] = tbl.reshape(128, ntiles, fdim)
        in_maps.append({
            "comb": comb.reshape(128, ntiles * TW),
            "wmat": w_bf,
            "ident": ident,
        })
    return in_maps, ntiles


_CACHE = {}


def kernel(f_atoms, W, func2atom, mapping, func_save_init, _trace=False):
    in_maps, ntiles = prepare_inputs(f_atoms, W, func2atom, mapping)
    if ntiles not in _CACHE:
        _CACHE[ntiles] = build_nc(ntiles)
    nc = _CACHE[ntiles]
    res = run_bass_kernel_spmd(nc, in_maps, list(range(N_CORES)),
                               trace=_trace)
    partial = sum(r["out"] for r in res.results)
    out = func_save_init.astype(np.float32) + partial.astype(np.float32)
    if _trace:
        kernel.last_exec_time_ns = res.exec_time_ns
    return out


# revision 16
# speedup vs baseline: 1.0351x; 1.0009x over previous
"""Trainium2 Bass kernel for CMPNEncoder functional-group embedding (v4).

out = func_save_init + A @ W,  A[s,:] = sum_a count_s[a] * f_atoms[a,:].

Per core (atoms sharded 8 ways): stream only the *referenced* atom rows
(~80% of the shard) paired with their per-segment count rows, reduce via
one fp8 matmul per 128-row tile into a PSUM [100,133] accumulator, then
apply W on-device.

v4 changes vs v3:
  - table rows shipped as fp8 e3m4 (1 B/elem, measured end-to-end rel err
    ~1.3e-2 < 2e-2 gate) instead of bf16 hi/lo pairs (4x fewer bytes,
    2x fewer matmuls).
  - counts shipped as fp8 e3m4 (exact for counts <= 32; measured max 3).
  - counts+table packed per tile into one DRAM tensor [128, ntiles*233]
    so each chunk is ONE large per-partition-contiguous DMA (>1 MiB =>
    near line rate, ~358 GB/s HBM/NC).
Host sums the per-core [100,300] partials (unshard) and adds
func_save_init.
"""

import sys

sys.path.insert(0, "/opt/trn_rl_repo")

import ml_dtypes
import numpy as np

import concourse.bacc as bacc
import concourse.mybir as mybir
from concourse.bass_utils import run_bass_kernel_spmd
from concourse.tile import TileContext

N_ATOMS = 400_000
FDIM = 133
HID = 300
NSEG = 100
N_CORES = 8
ROWS_PER_CORE = N_ATOMS // N_CORES
TW = 236                                  # padded tile-row slot (fp8 bytes)
TOFF = 100                                # table offset within slot (4B-aligned)
CHUNK = 64                                # tiles per streamed DMA chunk


def _round_up(x, m):
    return (x + m - 1) // m * m


def _chunk_sizes(ntiles, chunk=CHUNK):
    """Full-size chunks first, tapered at the END: the stream is DMA-bound,
    so total time ~= last-chunk-sem + PE work left after it. Small final
    chunks minimize that trailing PE work."""
    taper = [32, 16, 8]
    rem = ntiles - sum(taper)
    if rem <= 0:  # tiny problem fallback
        return [ntiles]
    sizes = [chunk] * (rem // chunk)
    if rem % chunk:
        sizes.append(rem % chunk)
    return sizes + taper


def build_nc(ntiles, fdim=FDIM, hid=HID, nseg=NSEG):
    f32, bf16, fp8 = (mybir.dt.float32, mybir.dt.bfloat16,
                      mybir.dt.float8e3)

    nc = bacc.Bacc("TRN2", target_bir_lowering=False, debug=False)

    comb = nc.declare_dram_parameter("comb", [128, ntiles * TW], fp8,
                                     isOutput=False)
    wmat = nc.declare_dram_parameter("wmat", [fdim, hid], bf16,
                                     isOutput=False)
    ident_d = nc.declare_dram_parameter("ident", [nseg, nseg], bf16,
                                        isOutput=False)
    out_d = nc.declare_dram_parameter("out", [nseg, hid], bf16,
                                      isOutput=True)

    sizes = _chunk_sizes(ntiles)

    with TileContext(nc) as tc:
        with (
            tc.tile_pool(name="const", bufs=1) as cpool,
            tc.tile_pool(name="stream", bufs=5) as spool,
            tc.tile_pool(name="psA", bufs=1, space="PSUM") as psA,
            tc.tile_pool(name="psT", bufs=1, space="PSUM") as psT,
            tc.tile_pool(name="sb2", bufs=1) as sb2,
        ):
            a_ps = psA.tile([nseg, fdim], f32, tag="A")

            # Issue ALL stream DMAs up front (the first one is the critical
            # path to the first matmul); consts (only needed at the epilogue)
            # go to the sync queue after the stream is rolling.
            chunks = []
            t0 = 0
            for g in sizes:
                ft = spool.tile([128, CHUNK * TW], fp8, tag="f")
                nc.sync.dma_start(out=ft[:, 0:g * TW],
                                  in_=comb[:, t0 * TW:(t0 + g) * TW])
                chunks.append((ft, g))
                t0 += g
                if len(chunks) == 1:
                    ident_t = cpool.tile([nseg, nseg], bf16, tag="ident")
                    nc.scalar.dma_start(out=ident_t[:, :], in_=ident_d[:, :])
                    wa_t = cpool.tile([128, hid], bf16, tag="wa")
                    nc.scalar.dma_start(out=wa_t[:, :], in_=wmat[0:128, :])
                    wb_t = cpool.tile([fdim - 128, hid], bf16, tag="wb")
                    nc.scalar.dma_start(out=wb_t[:, :], in_=wmat[128:fdim, :])

            # Split accumulation: A1 over the first `split` tiles gets its
            # whole transpose+W epilogue DURING the stream (PE slack);
            # only A2's short chain remains after the last chunk's DMA sem.
            split = sum(g for _, g in chunks[:-4])  # all but last ~4 chunks
            o_ps = psT.tile([nseg, hid], f32, tag="o")

            def half_epilogue(src_ps, first, last):
                a_sb = sb2.tile([nseg, fdim], bf16,
                                tag="a_sb%d" % int(first))
                nc.scalar.copy(out=a_sb[:, :], in_=src_ps[:, :])
                t1_ps = psT.tile([128, nseg], bf16, tag="t1%d" % int(first))
                nc.tensor.transpose(out=t1_ps[:, :], in_=a_sb[:, 0:128],
                                    identity=ident_t[:, :])
                at1_sb = sb2.tile([128, nseg], bf16,
                                  tag="at1%d" % int(first))
                nc.vector.tensor_copy(out=at1_sb[:, :], in_=t1_ps[:, :])
                t2_ps = psT.tile([fdim - 128, nseg], bf16,
                                 tag="t2%d" % int(first))
                nc.tensor.transpose(out=t2_ps[:, :], in_=a_sb[:, 128:fdim],
                                    identity=ident_t[:, :])
                at2_sb = sb2.tile([fdim - 128, nseg], bf16,
                                  tag="at2%d" % int(first))
                nc.scalar.copy(out=at2_sb[:, :], in_=t2_ps[:, :])
                nc.tensor.matmul(out=o_ps[:, :], lhsT=at1_sb[:, :],
                                 rhs=wa_t[:, :], start=first, stop=False)
                nc.tensor.matmul(out=o_ps[:, :], lhsT=at2_sb[:, :],
                                 rhs=wb_t[:, :], start=False, stop=last)

            a2_ps = psA.tile([nseg, fdim], f32, tag="A2")
            tglob = 0
            for ft, g in chunks:
                for j in range(g):
                    acc = a_ps if tglob < split else a2_ps
                    base = 0 if tglob < split else split
                    nc.tensor.matmul(
                        out=acc[:, :],
                        lhsT=ft[:, j * TW:j * TW + nseg],
                        rhs=ft[:, j * TW + TOFF:j * TW + TOFF + fdim],
                        start=(tglob == base),
                        stop=(tglob in (split - 1, ntiles - 1)),
                    )
                    tglob += 1
                    if tglob == split:
                        half_epilogue(a_ps, True, False)
            half_epilogue(a2_ps, False, True)

            o_sb = sb2.tile([nseg, hid], bf16, tag="o_sb")
            hh = hid // 2
            nc.vector.tensor_copy(out=o_sb[:, 0:hh], in_=o_ps[:, 0:hh])
            nc.scalar.copy(out=o_sb[:, hh:hid], in_=o_ps[:, hh:hid])
            nc.sync.dma_start(out=out_d[:, :], in_=o_sb[:, :])

    nc.compile()
    return nc


def prepare_inputs(f_atoms, W, func2atom, mapping,
                   n_cores=N_CORES, rows_tbl=ROWS_PER_CORE, nseg=NSEG):
    fdim = f_atoms.shape[1]
    flat = func2atom.astype(np.int64).ravel()
    seg = np.repeat(mapping.astype(np.int64), func2atom.shape[1])
    valid = flat > 0
    atom = flat[valid] - 1
    seg = seg[valid]
    core = atom // rows_tbl
    local = atom % rows_tbl

    # Per-core count matrices over the core's referenced (compacted) rows.
    percore = []
    for c in range(n_cores):
        m = core == c
        cnt = np.zeros((rows_tbl, nseg), dtype=np.float32)
        np.add.at(cnt, (local[m], seg[m]), 1.0)
        ref = np.flatnonzero(cnt.any(axis=1))
        percore.append((ref, cnt[ref]))

    rows_pad = _round_up(max(len(r) for r, _ in percore), 128)
    ntiles = rows_pad // 128
    ident = np.eye(nseg, dtype=ml_dtypes.bfloat16)
    w_bf = W.astype(ml_dtypes.bfloat16)

    in_maps = []
    for c in range(n_cores):
        ref, cnt = percore[c]
        n = len(ref)
        assert cnt.max() <= 32.0  # fp8 e3m4 is exact for small ints
        rows = f_atoms[c * rows_tbl:(c + 1) * rows_tbl][ref]
        comb = np.zeros((128, ntiles, TW), dtype=ml_dtypes.float8_e3m4)
        tbl = np.zeros((128 * ntiles, fdim), dtype=ml_dtypes.float8_e3m4)
        tbl[:n] = rows.astype(ml_dtypes.float8_e3m4)
        cp = np.zeros((128 * ntiles, nseg), dtype=ml_dtypes.float8_e3m4)
        cp[:n] = cnt.astype(ml_dtypes.float8_e3m4)
        # slot (p, t) holds compacted row p*ntiles + t so each partition's
        # DRAM stream is fully contiguous
        comb[:, :, :nseg] = cp.reshape(128, ntiles, nseg)
        comb[:, :, TOFF:TOFF + fdim] = tbl.reshape(128, ntiles, fdim)
        in_maps.append({
            "comb": comb.reshape(128, ntiles * TW),
            "wmat": w_bf,
            "ident": ident,
        })
    return in_maps, ntiles


_CACHE = {}


def kernel(f_atoms, W, func2atom, mapping, func_save_init, _trace=False):
    in_maps, ntiles = prepare_inputs(f_atoms, W, func2atom, mapping)
    if ntiles not in _CACHE:
        _CACHE[ntiles] = build_nc(ntiles)
    nc = _CACHE[ntiles]
    res = run_bass_kernel_spmd(nc, in_maps, list(range(N_CORES)),
                               trace=_trace)
    partial = sum(r["out"] for r in res.results)
    out = func_save_init.astype(np.float32) + partial.astype(np.float32)
    if _trace:
        kernel.last_exec_time_ns = res.exec_time_ns
    return out
